# revision 2
# baseline (speedup 1.0000x reference)
"""Block-diagonal matmul (BlockLinear) on 8 Trainium2 NeuronCores — bf16 I/O.

Problem: W [16, 64, 64] f32 stacked square blocks; inp [1024, 32768] f32.
out = block_diag(W) @ inp, i.e. per-block out[h] = W[h] @ inp[h*64:(h+1)*64, :].

Strategy (data parallel over the batch axis, per the sharding hint):
  - Shard inp / out along B=32768 across 8 cores (4096 columns each).
  - Host-side, cast to bf16 and pack the 16 64x64 blocks into 8
    block-diagonal 128x128 pairs, pre-transposed for the TensorE "lhsT"
    stationary operand (full 128-partition contraction, no device
    transposes). bf16 halves HBM traffic AND runs the PE at 1 cycle/row
    (f32 matmul is 4 cycles/row); accumulation stays f32 in PSUM, and the
    f32 result is rounded to bf16 on the PSUM->SBUF copy. End-to-end
    rounding error ~5e-3 max-normalized, well inside the 2e-2 gate.
  - Per core: for each of the 8 row-pairs, DMA a [128, 4096] bf16 slab in
    (1 MiB, HWDGE on the sync ring), run 8 matmuls of N=512 into PSUM f32,
    copy PSUM->SBUF bf16 on VectorE/ACT, DMA the result out on the
    scalar-engine HWDGE ring (separate FIFO from loads).

Memory-bound: 16.25 MiB HBM traffic per core ~= 47.6 us at the ~358 GB/s
per-core HBM cap (716 GB/s/stack shared by 2 NCs). The f32 predecessor
(32.25 MiB, PE at 4 cyc/row) measured ~102 us.
"""

import os
import sys

import numpy as np

for _p in ("/opt/trn_rl_repo", "/opt/pypackages"):
    if os.path.isdir(_p) and _p not in sys.path:
        sys.path.append(_p)

import ml_dtypes  # noqa: E402  (ships with jax; needed for numpy bf16)

BF16 = np.dtype(ml_dtypes.bfloat16)

H, D_BLK = 16, 64
D_TOTAL = H * D_BLK            # 1024
B = 32768
N_CORES = 8
BS = B // N_CORES              # 4096 batch columns per core
N_PAIR = H // 2                # 8 pairs of blocks -> 128 partitions each
FREE = 512                     # one PSUM bank of f32
NT = BS // FREE                # 8 matmuls per pair

_CACHE = {}


def _build_program(repeat: int = 1, variant: dict | None = None):
    import concourse.bacc as bacc
    import concourse.tile as tile
    from concourse import mybir

    # Defaults inherited from the f32-tuned baseline: deep double-buffering,
    # loads on the sync HWDGE ring, stores on the scalar ring (separate
    # FIFOs), last pair stored in finer chunks to shorten the tail,
    # PSUM->SBUF copies in 2-bank chunks split DVE(3)/ACT(1).
    v = dict(bufs_x=4, bufs_y=4, store_chunks=2, load_chunks=1,
             alt_engines=False, copy_act_from=6, last_sc=4,
             w_on_scalar=True, load_merge=1, copy_span=2,
             last_lc=None)
    v.update(variant or {})

    f32 = mybir.dt.float32
    bf16 = mybir.dt.bfloat16
    nc = bacc.Bacc("TRN2", target_bir_lowering=False, debug=False,
                   num_devices=N_CORES)

    w_d = nc.dram_tensor("w", (128, N_PAIR * 128), bf16, kind="ExternalInput")
    x_d = nc.dram_tensor("x", (N_PAIR, 128, BS), bf16, kind="ExternalInput")
    y_d = nc.dram_tensor("y", (N_PAIR, 128, BS), bf16, kind="ExternalOutput")

    with tile.TileContext(nc) as tc:
        with (
            tc.tile_pool(name="wpool", bufs=1) as wpool,
            tc.tile_pool(name="xpool", bufs=v["bufs_x"]) as xpool,
            tc.tile_pool(name="ypool", bufs=v["bufs_y"]) as ypool,
            tc.tile_pool(name="psum", bufs=8 // v["copy_span"],
                         space="PSUM") as psum_pool,
        ):
            wt = wpool.tile([128, N_PAIR * 128], bf16)
            (nc.scalar if v["w_on_scalar"] else nc.sync).dma_start(wt[:], w_d[:])

            x_r = x_d.rearrange("p k b -> k p b")
            y_r = y_d.rearrange("p k b -> k p b")

            def body():
                lc, lm = v["load_chunks"], v["load_merge"]
                for pg in range(N_PAIR // lm):
                    sc = v["store_chunks"]
                    my_lc = lc
                    if pg == N_PAIR // lm - 1:
                        if v["last_sc"]:
                            sc = v["last_sc"]
                        if v["last_lc"]:
                            my_lc = v["last_lc"]
                    if v["alt_engines"] and pg % 2:
                        ld_eng, st_eng = nc.scalar, nc.sync
                    else:
                        ld_eng, st_eng = nc.sync, nc.scalar
                    # xt holds lm pairs: [128, lm, BS]
                    xt = xpool.tile([128, lm, BS], bf16)
                    for i in range(my_lc):
                        w_ = BS // my_lc
                        ld_eng.dma_start(
                            xt[:, :, i * w_:(i + 1) * w_],
                            x_r[:, pg * lm:(pg + 1) * lm, i * w_:(i + 1) * w_])
                    yt = ypool.tile([128, lm, BS], bf16)
                    span = v["copy_span"]
                    for j in range(lm):
                        p = pg * lm + j
                        for n2 in range(NT // span):
                            ps = psum_pool.tile([128, span * FREE], f32)
                            for s in range(span):
                                n = n2 * span + s
                                nc.tensor.matmul(
                                    ps[:, s * FREE:(s + 1) * FREE],
                                    wt[:, p * 128:(p + 1) * 128],
                                    xt[:, j, n * FREE:(n + 1) * FREE],
                                    start=True, stop=True,
                                )
                            lo = n2 * span * FREE
                            hi = lo + span * FREE
                            if n2 * span >= v["copy_act_from"]:
                                nc.scalar.copy(yt[:, j, lo:hi], ps[:])
                            else:
                                nc.vector.tensor_copy(yt[:, j, lo:hi], ps[:])
                    for i in range(sc * lm):
                        w_ = BS // sc
                        j, ii = divmod(i, sc)
                        st_eng.dma_start(
                            y_r[:, pg * lm + j, ii * w_:(ii + 1) * w_],
                            yt[:, j, ii * w_:(ii + 1) * w_])

            if repeat == 1:
                body()
            else:
                with tc.For_i(0, repeat, 1):
                    body()

    nc.compile()
    return nc


def _get_program(repeat: int = 1, variant: dict | None = None):
    key = ("nc", repeat, tuple(sorted((variant or {}).items())))
    if key not in _CACHE:
        _CACHE[key] = _build_program(repeat, variant)
    return _CACHE[key]


def _pack_weights(W: np.ndarray) -> np.ndarray:
    """[16, 64, 64] f32 -> [128, 8*128] bf16 lhsT layout: col p*128+m, row k
    holds block_diag(W[2p].T, W[2p+1].T)[k, m]."""
    WD = np.zeros((N_PAIR, 128, 128), dtype=np.float32)
    for p in range(N_PAIR):
        WD[p, :D_BLK, :D_BLK] = W[2 * p].T
        WD[p, D_BLK:, D_BLK:] = W[2 * p + 1].T
    packed = np.ascontiguousarray(
        WD.transpose(1, 0, 2).reshape(128, N_PAIR * 128))
    return packed.astype(BF16)


def _pack_x(inp: np.ndarray) -> np.ndarray:
    """[1024, 32768] f32 -> [N_CORES*8, 128, BS] bf16; core c gets columns
    c*BS:(c+1)*BS."""
    xb = inp.astype(BF16)  # cast first so the transpose copy moves half the bytes
    return np.ascontiguousarray(
        xb.reshape(N_PAIR, 128, N_CORES, BS).transpose(2, 0, 1, 3)
    ).reshape(N_CORES * N_PAIR, 128, BS)


def _unpack_y(y_global: np.ndarray) -> np.ndarray:
    """[N_CORES*8, 128, BS] bf16 -> [1024, 32768] f32."""
    y = y_global.reshape(N_CORES, N_PAIR, 128, BS)
    return np.ascontiguousarray(
        y.transpose(1, 2, 0, 3)).reshape(D_TOTAL, B).astype(np.float32)


def _get_runner():
    """Build (once) the jitted 8-core dispatch for the bass program."""
    if "runner" in _CACHE:
        return _CACHE["runner"]

    import jax
    from concourse import mybir
    from concourse.bass2jax import (
        _bass_exec_p,
        install_neuronx_cc_hook,
        partition_id_tensor,
    )
    from jax.experimental.shard_map import shard_map
    from jax.sharding import Mesh, NamedSharding, PartitionSpec

    install_neuronx_cc_hook()
    nc = _get_program()

    partition_name = nc.partition_id_tensor.name if nc.partition_id_tensor else None
    in_names, out_names, out_avals, out_shapes = [], [], [], []
    for alloc in nc.m.functions[0].allocations:
        if not isinstance(alloc, mybir.MemoryLocationSet):
            continue
        name = alloc.memorylocations[0].name
        if alloc.kind == "ExternalInput":
            if name != partition_name:
                in_names.append(name)
        elif alloc.kind == "ExternalOutput":
            out_names.append(name)
            shape = tuple(alloc.tensor_shape)
            dtype = mybir.dt.np(alloc.dtype)
            out_avals.append(jax.core.ShapedArray(shape, dtype))
            out_shapes.append((shape, dtype))
    n_params = len(in_names)
    n_outs = len(out_avals)
    all_in_names = in_names + out_names
    if partition_name is not None:
        all_in_names.append(partition_name)
    donate = tuple(range(n_params, n_params + n_outs))

    def _body(*args):
        operands = list(args)
        if partition_name is not None:
            operands.append(partition_id_tensor())
        outs = _bass_exec_p.bind(
            *operands,
            out_avals=tuple(out_avals),
            in_names=tuple(all_in_names),
            out_names=tuple(out_names),
            lowering_input_output_aliases=(),
            sim_require_finite=True,
            sim_require_nnan=True,
            nc=nc,
        )
        return tuple(outs)

    devices = jax.devices()[:N_CORES]
    mesh = Mesh(np.asarray(devices), ("core",))
    in_specs = (PartitionSpec("core"),) * (n_params + n_outs)
    out_specs = (PartitionSpec("core"),) * n_outs
    sharded = jax.jit(
        shard_map(_body, mesh=mesh, in_specs=in_specs, out_specs=out_specs,
                  check_rep=False),
        donate_argnums=donate,
        keep_unused=True,
    )
    shard = NamedSharding(mesh, PartitionSpec("core"))

    import jax.numpy as jnp

    zero_shapes = [((shape[0] * N_CORES,) + shape[1:], dtype)
                   for shape, dtype in out_shapes]
    zeros_jit = jax.jit(
        lambda: tuple(jnp.zeros(s, d) for s, d in zero_shapes),
        out_shardings=tuple(shard for _ in zero_shapes),
    )

    def host_zeros():
        return [jax.device_put(np.zeros(s, d), shard) for s, d in zero_shapes]

    try:
        jax.block_until_ready(zeros_jit())
        make_zeros = lambda: list(zeros_jit())  # noqa: E731
    except Exception:
        make_zeros = host_zeros

    def run(global_ins: dict):
        """global_ins: name -> concatenated [N_CORES*dim0, ...] array."""
        dev_in = [jax.device_put(global_ins[name], shard)
                  for name in in_names]
        outs = sharded(*dev_in, *make_zeros())
        return {name: np.asarray(o) for name, o in zip(out_names, outs)}

    _CACHE["runner"] = run
    return run


def _kernel_direct(w_host: np.ndarray, inp: np.ndarray) -> np.ndarray:
    w_global = np.tile(w_host, (N_CORES, 1))
    x_global = _pack_x(inp)
    run = _get_runner()
    outs = run({"w": w_global, "x": x_global})
    return _unpack_y(outs["y"])


def _kernel_via_spmd(w_host: np.ndarray, inp: np.ndarray) -> np.ndarray:
    from concourse.bass_utils import run_bass_kernel_spmd

    nc = _get_program()
    x_global = _pack_x(inp)
    in_maps = []
    for c in range(N_CORES):
        in_maps.append({"w": w_host,
                        "x": x_global[c * N_PAIR:(c + 1) * N_PAIR]})
    res = run_bass_kernel_spmd(nc, in_maps, core_ids=list(range(N_CORES)))
    y_global = np.concatenate([np.asarray(res.results[c]["y"])
                               for c in range(N_CORES)], axis=0)
    return _unpack_y(y_global)


def kernel(W: np.ndarray, inp: np.ndarray) -> np.ndarray:
    W = np.asarray(W, dtype=np.float32)
    inp = np.asarray(inp, dtype=np.float32)
    assert W.shape == (H, D_BLK, D_BLK) and inp.shape == (D_TOTAL, B)

    w_host = _pack_weights(W)

    try:
        from concourse._compat import axon_active
        use_direct = axon_active()
    except Exception:
        use_direct = False

    if use_direct:
        try:
            return _kernel_direct(w_host, inp)
        except Exception:
            # Transient device wedges (NRT_EXEC_UNIT_UNRECOVERABLE) have been
            # observed to need ~60 s to clear; retry once after a long
            # backoff, then fall back to the run_bass_kernel_spmd path.
            import time
            time.sleep(45)
            try:
                return _kernel_direct(w_host, inp)
            except Exception:
                time.sleep(30)
    return _kernel_via_spmd(w_host, inp)


if __name__ == "__main__":
    rng = np.random.default_rng(0)
    W = rng.standard_normal((H, D_BLK, D_BLK), dtype=np.float32)
    inp = rng.standard_normal((D_TOTAL, B), dtype=np.float32)
    out = kernel(W, inp)
    ref = np.einsum("hij,hjb->hib", W, inp.reshape(H, D_BLK, B)).reshape(D_TOTAL, B)
    err = np.abs(out - ref).max() / max(np.abs(ref).max(), 1e-9)
    print("self-check rel err:", err)
    assert err < 2e-2, err
